# revision 31
# baseline (speedup 1.0000x reference)
"""BinarizeLinear Trainium2 kernel.

Computes y = binarize(x) @ binarize(W)^T + bias where binarize(t) = where(t>0, +1, -1),
x: [8192, 4096] f32, W: [4096, 4096] f32, bias: [4096] f32.

Strategy (8 NeuronCores, data parallel over tokens):
  - Each core gets 1024 tokens: xT shard [D_IN, 1024] (host-transposed layout so the
    contraction dim lands on SBUF partitions), the full W^T [D_IN, D_OUT], and bias.
  - On device: binarize via ScalarE Sign(t - 1e-20) (exact where(t>0,+1,-1) semantics,
    incl. t==0 -> -1) into fp8e4m3 (+/-1 exact in fp8; accumulation is fp32 PSUM so the
    whole matmul is exact). x kept resident in SBUF as fp8; W streamed fp32 -> fp8.
  - TensorE matmul in DoubleRow fp8 mode (2 MACs/cell/cycle), out^T[o, m] tiles in PSUM,
    bias added during the PSUM->SBUF drain on VectorE (bias is per-partition in this
    orientation), then DMA to DRAM as outT [D_OUT, 1024].
  - Host gathers the 8 outT shards and transposes back to [8192, 4096].
"""

import os
import sys

import numpy as np

sys.path.insert(0, "/opt/trn_rl_repo")

import concourse.bacc as bacc
import concourse.mybir as mybir
import concourse.tile as tile
from concourse.bass import ds, ts
from concourse.bass_utils import run_bass_kernel_spmd
from concourse.kernels.tile_matmul import (
    ShapeInfo,
    composable_matmul_tile_kernel,
)

N_TOK, D_IN, D_OUT = 8192, 4096, 4096
NCORES = 8
M_LOC = N_TOK // NCORES  # tokens per core
P = 128
KT = 4  # K subtiles per K tile (K_TILE = 512)
NEG_TINY = -1.0e-20  # Sign(t + NEG_TINY): t>0 -> +1, t<=0 -> -1

F32 = mybir.dt.float32
BF16 = mybir.dt.bfloat16
FP8 = mybir.dt.float8e4
SIGN = mybir.ActivationFunctionType.Sign
IDENT = mybir.ActivationFunctionType.Identity

LAST_EXEC_TIME_NS = None


def build(d_in=D_IN, d_out=D_OUT, m_loc=M_LOC):
    """Build the per-core Bass program (SPMD: all cores run the same NEFF)."""
    xbin = os.environ.get("BINLIN_XBIN", "dve")  # dve | act
    drain = os.environ.get("BINLIN_DRAIN", "mixed")  # mixed | dve
    out_eng = os.environ.get("BINLIN_OUT", "gpsimd")  # gpsimd | sync
    k_tiles = d_in // (P * KT)
    nc = bacc.Bacc("TRN2", target_bir_lowering=False, debug=False)

    xT = nc.dram_tensor("xT", (d_in, m_loc), F32, kind="ExternalInput")
    wT = nc.dram_tensor("wT", (d_in, d_out), F32, kind="ExternalInput")
    bias = nc.dram_tensor("bias", (d_out,), F32, kind="ExternalInput")
    outT = nc.dram_tensor("outT", (d_out, m_loc), F32, kind="ExternalOutput")

    with tile.TileContext(nc) as tc:
        with (
            tc.tile_pool(name="const", bufs=1) as const,
            tc.tile_pool(name="xstage", bufs=2) as xstage,
            tc.tile_pool(name="xb", bufs=1) as xbpool,
            tc.tile_pool(name="wstage", bufs=4) as wstage,
            tc.tile_pool(name="kxm", bufs=k_tiles + 1) as kxm_pool,
            tc.tile_pool(name="partial", bufs=1) as partial_pool,
        ):
            # bias_sb[p, c] = bias[c*128 + p]. A direct strided gather would be 4096
            # 4-byte DMA descriptors and hogs a HWDGE ring for ~18us; instead load
            # bias as [R, 128] (R contiguous 512B rows) and block-transpose on DVE.
            R = d_out // P
            assert R <= 32
            bias_row = const.tile([32, P], F32)
            if R < 32:
                nc.any.memset(bias_row[:], 0.0)
            nc.scalar.dma_start(bias_row[:R, :], bias[:].rearrange("(r s) -> r s", s=P))
            bias_sb = const.tile([P, 32], F32)
            for i in range(4):
                nc.vector.transpose(
                    bias_sb[32 * i : 32 * i + 32, 0:32],
                    bias_row[:, 32 * i : 32 * i + 32],
                )

            # Per-partition bias for Sign(t + NEG_TINY) (float biases need a const AP)
            sign_bias = const.tile([P, 1], F32)
            nc.any.memset(sign_bias[:], NEG_TINY)

            xT_t = xT[:].rearrange("(po pi) m -> pi po m", pi=P)  # [128, d_in/128, m]
            wT_t = wT[:].rearrange("(po pi) o -> pi po o", pi=P)  # [128, d_in/128, o]

            # Load + binarize x once; keep resident in SBUF as fp8 (one tile per K_TILE
            # so matmul dependencies are per-K-tile, letting PE start before the full
            # load finishes). Engine routing is the whole game here:
            #   - x DMAs ride the GpSimd SWDGE ring: putting them on a HWDGE ring
            #     head-of-line-blocks that sequencer's other work (the x dispatches
            #     wait on stage-buffer recycling), which measured as a ~65us PE stall.
            #   - W loads own the sync ring, W binarize owns ACT.
            #   - x binarize runs on VectorE as two exact tensor_scalar ops:
            #     u = (t > 0) in {0,1}, then 2u - 1 in {-1,+1}.
            xb_tiles = [
                xbpool.tile([P, KT, m_loc], FP8, tag=f"xb{kt}", name=f"xb{kt}")
                for kt in range(k_tiles)
            ]
            if xbin == "dve":
                # One 2MB chunk per xb tile: fewer SWDGE dma_starts (Q7 descriptor
                # generation paces the x stream at 1MB granularity).
                for kt in range(k_tiles):
                    stg = xstage.tile([P, KT, m_loc], F32, tag="xstage")
                    nc.gpsimd.dma_start(stg[:], xT_t[:, ts(kt, KT), :])
                    xu = xstage.tile([P, KT, m_loc], BF16, tag="xu")
                    nc.vector.tensor_scalar(
                        xu[:], stg[:], 0.0, None, mybir.AluOpType.is_gt
                    )
                    nc.vector.tensor_scalar(
                        xb_tiles[kt][:],
                        xu[:], 2.0, -1.0,
                        mybir.AluOpType.mult, mybir.AluOpType.add,
                    )
            else:
                for kt in range(k_tiles):
                    stg = xstage.tile([P, KT, m_loc], F32, tag="xstage")
                    nc.scalar.dma_start(stg[:], xT_t[:, ts(kt, KT), :])
                    nc.scalar.activation(
                        xb_tiles[kt][:], stg[:], SIGN, bias=sign_bias[:]
                    )

            def kxn_producer(nc_, md):
                return xb_tiles[md.k_tile_idx][:, :, ts(md.n_tile_idx, md.n_tile)]

            # W producer with software prefetch-one-ahead: emitting the next tile's
            # DMA + Sign during the current call places them early in the sync/ACT
            # FIFOs, hiding the ~6us produce latency that otherwise stalls the PE at
            # every m-tile boundary (each such gap also re-throttles the PE clock).
            m_tiles_w = d_out // 512
            PRE_M = 2  # m-tiles computed in two K-halves during the x-load phase
            KH = k_tiles // 2
            prepass = (
                k_tiles % 2 == 0
                and m_tiles_w > PRE_M
                and os.environ.get("BINLIN_PREPASS", "1") == "1"
            )
            if prepass:
                worder = (
                    [(m, k) for m in range(PRE_M) for k in range(KH)]
                    + [(m, k + KH) for m in range(PRE_M) for k in range(KH)]
                    + [(m, k) for m in range(PRE_M, m_tiles_w) for k in range(k_tiles)]
                )
            else:
                worder = [(m, k) for m in range(m_tiles_w) for k in range(k_tiles)]
            wpos = {mk: i for i, mk in enumerate(worder)}
            wcache = {}

            def _produce_w(nc_, m_idx, k_idx):
                stg = wstage.tile([P, KT, 512], F32, tag="wstage", name="wstage")
                nc_.sync.dma_start(
                    stg[:], wT_t[:, ts(k_idx, KT), ts(m_idx, 512)]
                )
                wb = kxm_pool.tile([P, KT, 512], FP8, tag="wb", name="wb")
                nc_.scalar.activation(wb[:], stg[:], SIGN, bias=sign_bias[:])
                return wb

            def make_kxm_producer(m_off, k_off):
                def kxm_producer(nc_, md):
                    mk = (md.m_tile_idx + m_off, md.k_tile_idx + k_off)
                    if mk not in wcache:
                        wcache[mk] = _produce_w(nc_, *mk)
                    nxt = wpos[mk] + 1
                    if nxt < len(worder) and worder[nxt] not in wcache:
                        wcache[worder[nxt]] = _produce_w(nc_, *worder[nxt])
                    return wcache.pop(mk)

                return kxm_producer

            def make_kxn_producer(k_off):
                def kxn_producer(nc_, md):
                    return xb_tiles[md.k_tile_idx + k_off][
                        :, :, ts(md.n_tile_idx, md.n_tile)
                    ]

                return kxn_producer

            def make_reducer(m_off):
                def reducer(nc_, psum, sbuf, md):
                    # psum: [128(o), FREE]; bias is per-partition here (DVE drain;
                    # ACT stays dedicated to W binarize).
                    col = (md.m_tile_idx + m_off) * (md.m_tile // P) + md.m_subtile_idx
                    nc_.vector.tensor_scalar(
                        sbuf,
                        psum,
                        bias_sb[:, col : col + 1],
                        None,
                        mybir.AluOpType.add,
                    )

                return reducer

            # Output stores ride the (otherwise idle) GpSimd SWDGE ring so they never
            # head-of-line-block the W loads on the sync ring.
            outT_t = outT[:].rearrange("(po pi) f -> pi po f", pi=P)

            def make_consumer(m_off):
                def consumer(nc_, mxn_tile, md):
                    eng = nc_.gpsimd if out_eng == "gpsimd" else nc_.sync
                    eng.dma_start(
                        outT_t[
                            :,
                            ts(md.m_tile_idx + m_off, md.m_subtiles),
                            ds(md.n_tile_idx * md.n_tile, md.n_slice_size),
                        ],
                        mxn_tile[:, :, : md.n_slice_size],
                    )

                return consumer

            common = dict(
                tc=tc,
                output_type=F32,
                MATMUL_FREE_DIM=512,
                MAX_TILE_SIZE=512,
                MAX_K_TILE_SIZE=P * KT,
                cache_tiles=True,
                temps_n_bufs=4,
                psum_n_bufs=2,
            )

            if prepass:
                # Split-K pre-pass for the first PRE_M m-tiles: while x is still
                # streaming in, the PE can only accumulate into 8 PSUM banks per
                # pass over K; splitting K in half doubles the PE work available
                # during the x-load window. Half A drains psum+bias to SBUF
                # partials; half B adds psum+partial and stores.
                half_shape_m = ShapeInfo(pdims=((P, KH * KT),), fdims=(PRE_M * 512,))
                half_shape_n = ShapeInfo(pdims=((P, KH * KT),), fdims=(m_loc,))
                partials = {}

                def partial_producer(nc_, md):
                    key = (md.m_tile_idx, md.n_tile_idx)
                    t = partial_pool.tile(
                        [P, md.m_subtiles, md.n_tile], F32,
                        tag=f"part{key}", name=f"part{key}",
                    )
                    partials[key] = t
                    return t

                def reducer_a(nc_, psum, sbuf, md):
                    # Partial K-half sums are exact integers; keep them bias-free so
                    # the final result rounds identically to the reference's
                    # (full integer sum) + bias.
                    nc_.vector.tensor_copy(sbuf, psum)

                def reducer_b(nc_, psum, sbuf, md):
                    part = partials[(md.m_tile_idx, md.n_tile_idx)]
                    nc_.vector.tensor_tensor(
                        sbuf,
                        psum,
                        part[
                            :,
                            md.m_subtile_idx : md.m_subtile_idx + 1,
                            ts(md.n_subtile_idx, md.n_subtile),
                        ],
                        mybir.AluOpType.add,
                    )
                    col = md.m_tile_idx * (md.m_tile // P) + md.m_subtile_idx
                    nc_.vector.tensor_scalar(
                        sbuf, sbuf, bias_sb[:, col : col + 1], None,
                        mybir.AluOpType.add,
                    )

                # The fill-phase calls run x-gated, so their drains hide inside the
                # x-arrival gaps even with psum_n_bufs=1 — and using only 4 banks
                # keeps the other 4 free so the next call's first matmuls don't
                # alias-wait on this call's final drains (measured ~12us at A->B).
                fill_common = {**common, "psum_n_bufs": 1}
                composable_matmul_tile_kernel(
                    kxm_shape=half_shape_m,
                    kxn_shape=half_shape_n,
                    kxm_producer=make_kxm_producer(0, 0),
                    kxn_producer=make_kxn_producer(0),
                    mxn_consumer=lambda nc_, t, md: None,
                    mxn_subtile_reducer=reducer_a,
                    mxn_subtile_producer=partial_producer,
                    **fill_common,
                )
                composable_matmul_tile_kernel(
                    kxm_shape=half_shape_m,
                    kxn_shape=half_shape_n,
                    kxm_producer=make_kxm_producer(0, KH),
                    kxn_producer=make_kxn_producer(KH),
                    mxn_consumer=make_consumer(0),
                    mxn_subtile_reducer=reducer_b,
                    **fill_common,
                )
                composable_matmul_tile_kernel(
                    kxm_shape=ShapeInfo(
                        pdims=((P, d_in // P),), fdims=(d_out - PRE_M * 512,)
                    ),
                    kxn_shape=ShapeInfo(pdims=((P, d_in // P),), fdims=(m_loc,)),
                    kxm_producer=make_kxm_producer(PRE_M, 0),
                    kxn_producer=make_kxn_producer(0),
                    mxn_consumer=make_consumer(PRE_M),
                    mxn_subtile_reducer=make_reducer(PRE_M),
                    **common,
                )
            else:
                composable_matmul_tile_kernel(
                    kxm_shape=ShapeInfo(pdims=((P, d_in // P),), fdims=(d_out,)),
                    kxn_shape=ShapeInfo(pdims=((P, d_in // P),), fdims=(m_loc,)),
                    kxm_producer=make_kxm_producer(0, 0),
                    kxn_producer=make_kxn_producer(0),
                    mxn_consumer=make_consumer(0),
                    mxn_subtile_reducer=make_reducer(0),
                    **common,
                )

    nc.compile()
    return nc


_NC_CACHE = None


def _get_nc():
    global _NC_CACHE
    if _NC_CACHE is None:
        _NC_CACHE = build()
    return _NC_CACHE


def _ensure_ntff_hook():
    """Register the axon NTFF profile hook if the image's antenv lacks it.

    bass_utils reads the hook via antenv.axon_hooks; this container's antenv has
    no axon_hooks module, but the slim boot package ships the ctypes equivalent.
    """
    try:
        from antenv.axon_hooks import get_axon_ntff_profile_hook  # noqa: F401

        return True
    except ImportError:
        pass
    try:
        import types

        import antenv
        from trn_agent_boot.trn_boot import _ntff_profile_via_ctypes

        hook = _ntff_profile_via_ctypes("/opt/axon/libaxon_pjrt.so")
        if hook is None:
            return False
        mod = types.ModuleType("antenv.axon_hooks")
        state = {"hook": hook}
        mod.set_axon_ntff_profile_hook = lambda h: state.update(hook=h)
        mod.get_axon_ntff_profile_hook = lambda: state["hook"]
        sys.modules["antenv.axon_hooks"] = mod
        antenv.axon_hooks = mod
        return True
    except Exception as e:  # profiling is best-effort; never block execution
        print(f"NTFF hook setup failed ({type(e).__name__}: {e}); no HW timing",
              file=sys.stderr)
        return False


def kernel(x, weight, bias):
    global LAST_EXEC_TIME_NS
    x = np.ascontiguousarray(np.asarray(x, dtype=np.float32))
    weight = np.asarray(weight, dtype=np.float32)
    bias = np.ascontiguousarray(np.asarray(bias, dtype=np.float32))
    wT = np.ascontiguousarray(weight.T)

    in_maps = []
    for c in range(NCORES):
        xT_c = np.ascontiguousarray(x[c * M_LOC : (c + 1) * M_LOC, :].T)
        in_maps.append({"xT": xT_c, "wT": wT, "bias": bias})

    nc = _get_nc()
    trace = os.environ.get("BINLIN_TRACE", "0") == "1"
    if trace:
        trace = _ensure_ntff_hook()
    core_ids = list(range(NCORES))
    if trace:
        try:
            res = run_bass_kernel_spmd(nc, in_maps, core_ids=core_ids, trace=True)
        except Exception as e:
            print(f"traced run failed ({type(e).__name__}: {e}); retry untraced",
                  file=sys.stderr)
            res = run_bass_kernel_spmd(nc, in_maps, core_ids=core_ids, trace=False)
    else:
        res = run_bass_kernel_spmd(nc, in_maps, core_ids=core_ids, trace=False)
    if res.exec_time_ns is not None:
        LAST_EXEC_TIME_NS = res.exec_time_ns

    outT = np.empty((D_OUT, N_TOK), dtype=np.float32)
    for c in range(NCORES):
        outT[:, c * M_LOC : (c + 1) * M_LOC] = res.results[c]["outT"]
    return np.ascontiguousarray(outT.T)


# revision 32
# speedup vs baseline: 1.1519x; 1.1519x over previous
"""BinarizeLinear Trainium2 kernel.

Computes y = binarize(x) @ binarize(W)^T + bias where binarize(t) = where(t>0, +1, -1),
x: [8192, 4096] f32, W: [4096, 4096] f32, bias: [4096] f32.

Strategy (8 NeuronCores, data parallel over tokens):
  - Each core gets 1024 tokens: xT shard [D_IN, 1024] (host-transposed layout so the
    contraction dim lands on SBUF partitions), the full W^T [D_IN, D_OUT], and bias.
  - On device: binarize via ScalarE Sign(t - 1e-20) (exact where(t>0,+1,-1) semantics,
    incl. t==0 -> -1) into fp8e4m3 (+/-1 exact in fp8; accumulation is fp32 PSUM so the
    whole matmul is exact). x kept resident in SBUF as fp8; W streamed fp32 -> fp8.
  - TensorE matmul in DoubleRow fp8 mode (2 MACs/cell/cycle), out^T[o, m] tiles in PSUM,
    bias added during the PSUM->SBUF drain on VectorE (bias is per-partition in this
    orientation), then DMA to DRAM as outT [D_OUT, 1024].
  - Host gathers the 8 outT shards and transposes back to [8192, 4096].
"""

import os
import sys

import numpy as np

sys.path.insert(0, "/opt/trn_rl_repo")

import concourse.bacc as bacc
import concourse.mybir as mybir
import concourse.tile as tile
from concourse.bass import ds, ts
from concourse.bass_utils import run_bass_kernel_spmd
from concourse.kernels.tile_matmul import (
    ShapeInfo,
    composable_matmul_tile_kernel,
)

N_TOK, D_IN, D_OUT = 8192, 4096, 4096
NCORES = 8
M_LOC = N_TOK // NCORES  # tokens per core
P = 128
KT = 4  # K subtiles per K tile (K_TILE = 512)
NEG_TINY = -1.0e-20  # Sign(t + NEG_TINY): t>0 -> +1, t<=0 -> -1

F32 = mybir.dt.float32
BF16 = mybir.dt.bfloat16
FP8 = mybir.dt.float8e4
SIGN = mybir.ActivationFunctionType.Sign
IDENT = mybir.ActivationFunctionType.Identity

LAST_EXEC_TIME_NS = None


def build(d_in=D_IN, d_out=D_OUT, m_loc=M_LOC):
    """Build the per-core Bass program (SPMD: all cores run the same NEFF)."""
    xbin = os.environ.get("BINLIN_XBIN", "dve")  # dve | act
    drain = os.environ.get("BINLIN_DRAIN", "mixed")  # mixed | dve
    out_eng = os.environ.get("BINLIN_OUT", "gpsimd")  # gpsimd | sync
    k_tiles = d_in // (P * KT)
    nc = bacc.Bacc("TRN2", target_bir_lowering=False, debug=False)

    xT = nc.dram_tensor("xT", (d_in, m_loc), F32, kind="ExternalInput")
    wT = nc.dram_tensor("wT", (d_in, d_out), F32, kind="ExternalInput")
    bias = nc.dram_tensor("bias", (d_out,), F32, kind="ExternalInput")
    outT = nc.dram_tensor("outT", (d_out, m_loc), F32, kind="ExternalOutput")

    with tile.TileContext(nc) as tc:
        with (
            tc.tile_pool(name="const", bufs=1) as const,
            tc.tile_pool(name="xstage", bufs=4) as xstage,
            tc.tile_pool(name="xb", bufs=1) as xbpool,
            tc.tile_pool(name="wstage", bufs=4) as wstage,
            tc.tile_pool(name="kxm", bufs=k_tiles + 1) as kxm_pool,
            tc.tile_pool(name="partial", bufs=1) as partial_pool,
        ):
            # bias_sb[p, c] = bias[c*128 + p]. A direct strided gather would be 4096
            # 4-byte DMA descriptors and hogs a HWDGE ring for ~18us; instead load
            # bias as [R, 128] (R contiguous 512B rows) and block-transpose on DVE.
            R = d_out // P
            assert R <= 32
            bias_row = const.tile([32, P], F32)
            if R < 32:
                nc.any.memset(bias_row[:], 0.0)
            nc.scalar.dma_start(bias_row[:R, :], bias[:].rearrange("(r s) -> r s", s=P))
            bias_sb = const.tile([P, 32], F32)
            for i in range(4):
                nc.vector.transpose(
                    bias_sb[32 * i : 32 * i + 32, 0:32],
                    bias_row[:, 32 * i : 32 * i + 32],
                )

            # Per-partition bias for Sign(t + NEG_TINY) (float biases need a const AP)
            sign_bias = const.tile([P, 1], F32)
            nc.any.memset(sign_bias[:], NEG_TINY)

            xT_t = xT[:].rearrange("(po pi) m -> pi po m", pi=P)  # [128, d_in/128, m]
            wT_t = wT[:].rearrange("(po pi) o -> pi po o", pi=P)  # [128, d_in/128, o]

            # Load + binarize x once; keep resident in SBUF as fp8 (one tile per K_TILE
            # so matmul dependencies are per-K-tile, letting PE start before the full
            # load finishes). Engine routing is the whole game here:
            #   - x DMAs ride the GpSimd SWDGE ring: putting them on a HWDGE ring
            #     head-of-line-blocks that sequencer's other work (the x dispatches
            #     wait on stage-buffer recycling), which measured as a ~65us PE stall.
            #   - W loads own the sync ring, W binarize owns ACT.
            #   - x binarize runs on VectorE as two exact tensor_scalar ops:
            #     u = (t > 0) in {0,1}, then 2u - 1 in {-1,+1}.
            xb_tiles = [
                xbpool.tile([P, KT, m_loc], FP8, tag=f"xb{kt}", name=f"xb{kt}")
                for kt in range(k_tiles)
            ]
            if xbin == "dve":
                for c in range(d_in // (P * 2)):
                    stg = xstage.tile([P, 2, m_loc], F32, tag="xstage")
                    nc.gpsimd.dma_start(stg[:], xT_t[:, ts(c, 2), :])
                    xu = xstage.tile([P, 2, m_loc], BF16, tag="xu")
                    nc.vector.tensor_scalar(
                        xu[:], stg[:], 0.0, None, mybir.AluOpType.is_gt
                    )
                    kt, half = divmod(c, 2)
                    nc.vector.tensor_scalar(
                        xb_tiles[kt][:, 2 * half : 2 * half + 2, :],
                        xu[:], 2.0, -1.0,
                        mybir.AluOpType.mult, mybir.AluOpType.add,
                    )
            else:
                for kt in range(k_tiles):
                    stg = xstage.tile([P, KT, m_loc], F32, tag="xstage")
                    nc.scalar.dma_start(stg[:], xT_t[:, ts(kt, KT), :])
                    nc.scalar.activation(
                        xb_tiles[kt][:], stg[:], SIGN, bias=sign_bias[:]
                    )

            def kxn_producer(nc_, md):
                return xb_tiles[md.k_tile_idx][:, :, ts(md.n_tile_idx, md.n_tile)]

            # W producer with software prefetch-one-ahead: emitting the next tile's
            # DMA + Sign during the current call places them early in the sync/ACT
            # FIFOs, hiding the ~6us produce latency that otherwise stalls the PE at
            # every m-tile boundary (each such gap also re-throttles the PE clock).
            m_tiles_w = d_out // 512
            PRE_M = 2  # m-tiles computed in two K-halves during the x-load phase
            KH = k_tiles // 2
            prepass = (
                k_tiles % 2 == 0
                and m_tiles_w > PRE_M
                and os.environ.get("BINLIN_PREPASS", "1") == "1"
            )
            if prepass:
                worder = (
                    [(m, k) for m in range(PRE_M) for k in range(KH)]
                    + [(m, k + KH) for m in range(PRE_M) for k in range(KH)]
                    + [(m, k) for m in range(PRE_M, m_tiles_w) for k in range(k_tiles)]
                )
            else:
                worder = [(m, k) for m in range(m_tiles_w) for k in range(k_tiles)]
            wpos = {mk: i for i, mk in enumerate(worder)}
            wcache = {}

            def _produce_w(nc_, m_idx, k_idx):
                stg = wstage.tile([P, KT, 512], F32, tag="wstage", name="wstage")
                nc_.sync.dma_start(
                    stg[:], wT_t[:, ts(k_idx, KT), ts(m_idx, 512)]
                )
                wb = kxm_pool.tile([P, KT, 512], FP8, tag="wb", name="wb")
                nc_.scalar.activation(wb[:], stg[:], SIGN, bias=sign_bias[:])
                return wb

            def make_kxm_producer(m_off, k_off):
                def kxm_producer(nc_, md):
                    mk = (md.m_tile_idx + m_off, md.k_tile_idx + k_off)
                    if mk not in wcache:
                        wcache[mk] = _produce_w(nc_, *mk)
                    nxt = wpos[mk] + 1
                    if nxt < len(worder) and worder[nxt] not in wcache:
                        wcache[worder[nxt]] = _produce_w(nc_, *worder[nxt])
                    return wcache.pop(mk)

                return kxm_producer

            def make_kxn_producer(k_off):
                def kxn_producer(nc_, md):
                    return xb_tiles[md.k_tile_idx + k_off][
                        :, :, ts(md.n_tile_idx, md.n_tile)
                    ]

                return kxn_producer

            def make_reducer(m_off):
                def reducer(nc_, psum, sbuf, md):
                    # psum: [128(o), FREE]; bias is per-partition here (DVE drain;
                    # ACT stays dedicated to W binarize).
                    col = (md.m_tile_idx + m_off) * (md.m_tile // P) + md.m_subtile_idx
                    nc_.vector.tensor_scalar(
                        sbuf,
                        psum,
                        bias_sb[:, col : col + 1],
                        None,
                        mybir.AluOpType.add,
                    )

                return reducer

            # Output stores ride the (otherwise idle) GpSimd SWDGE ring so they never
            # head-of-line-block the W loads on the sync ring.
            outT_t = outT[:].rearrange("(po pi) f -> pi po f", pi=P)

            def make_consumer(m_off):
                def consumer(nc_, mxn_tile, md):
                    eng = nc_.gpsimd if out_eng == "gpsimd" else nc_.sync
                    eng.dma_start(
                        outT_t[
                            :,
                            ts(md.m_tile_idx + m_off, md.m_subtiles),
                            ds(md.n_tile_idx * md.n_tile, md.n_slice_size),
                        ],
                        mxn_tile[:, :, : md.n_slice_size],
                    )

                return consumer

            common = dict(
                tc=tc,
                output_type=F32,
                MATMUL_FREE_DIM=512,
                MAX_TILE_SIZE=512,
                MAX_K_TILE_SIZE=P * KT,
                cache_tiles=True,
                temps_n_bufs=4,
                psum_n_bufs=2,
            )

            if prepass:
                # Split-K pre-pass for the first PRE_M m-tiles: while x is still
                # streaming in, the PE can only accumulate into 8 PSUM banks per
                # pass over K; splitting K in half doubles the PE work available
                # during the x-load window. Half A drains psum+bias to SBUF
                # partials; half B adds psum+partial and stores.
                half_shape_m = ShapeInfo(pdims=((P, KH * KT),), fdims=(PRE_M * 512,))
                half_shape_n = ShapeInfo(pdims=((P, KH * KT),), fdims=(m_loc,))
                partials = {}

                def partial_producer(nc_, md):
                    key = (md.m_tile_idx, md.n_tile_idx)
                    t = partial_pool.tile(
                        [P, md.m_subtiles, md.n_tile], F32,
                        tag=f"part{key}", name=f"part{key}",
                    )
                    partials[key] = t
                    return t

                def reducer_a(nc_, psum, sbuf, md):
                    # Partial K-half sums are exact integers; keep them bias-free so
                    # the final result rounds identically to the reference's
                    # (full integer sum) + bias.
                    nc_.vector.tensor_copy(sbuf, psum)

                def reducer_b(nc_, psum, sbuf, md):
                    part = partials[(md.m_tile_idx, md.n_tile_idx)]
                    nc_.vector.tensor_tensor(
                        sbuf,
                        psum,
                        part[
                            :,
                            md.m_subtile_idx : md.m_subtile_idx + 1,
                            ts(md.n_subtile_idx, md.n_subtile),
                        ],
                        mybir.AluOpType.add,
                    )
                    col = md.m_tile_idx * (md.m_tile // P) + md.m_subtile_idx
                    nc_.vector.tensor_scalar(
                        sbuf, sbuf, bias_sb[:, col : col + 1], None,
                        mybir.AluOpType.add,
                    )

                composable_matmul_tile_kernel(
                    kxm_shape=half_shape_m,
                    kxn_shape=half_shape_n,
                    kxm_producer=make_kxm_producer(0, 0),
                    kxn_producer=make_kxn_producer(0),
                    mxn_consumer=lambda nc_, t, md: None,
                    mxn_subtile_reducer=reducer_a,
                    mxn_subtile_producer=partial_producer,
                    **common,
                )
                composable_matmul_tile_kernel(
                    kxm_shape=half_shape_m,
                    kxn_shape=half_shape_n,
                    kxm_producer=make_kxm_producer(0, KH),
                    kxn_producer=make_kxn_producer(KH),
                    mxn_consumer=make_consumer(0),
                    mxn_subtile_reducer=reducer_b,
                    **common,
                )
                composable_matmul_tile_kernel(
                    kxm_shape=ShapeInfo(
                        pdims=((P, d_in // P),), fdims=(d_out - PRE_M * 512,)
                    ),
                    kxn_shape=ShapeInfo(pdims=((P, d_in // P),), fdims=(m_loc,)),
                    kxm_producer=make_kxm_producer(PRE_M, 0),
                    kxn_producer=make_kxn_producer(0),
                    mxn_consumer=make_consumer(PRE_M),
                    mxn_subtile_reducer=make_reducer(PRE_M),
                    **common,
                )
            else:
                composable_matmul_tile_kernel(
                    kxm_shape=ShapeInfo(pdims=((P, d_in // P),), fdims=(d_out,)),
                    kxn_shape=ShapeInfo(pdims=((P, d_in // P),), fdims=(m_loc,)),
                    kxm_producer=make_kxm_producer(0, 0),
                    kxn_producer=make_kxn_producer(0),
                    mxn_consumer=make_consumer(0),
                    mxn_subtile_reducer=make_reducer(0),
                    **common,
                )

    nc.compile()
    return nc


_NC_CACHE = None


def _get_nc():
    global _NC_CACHE
    if _NC_CACHE is None:
        _NC_CACHE = build()
    return _NC_CACHE


def _ensure_ntff_hook():
    """Register the axon NTFF profile hook if the image's antenv lacks it.

    bass_utils reads the hook via antenv.axon_hooks; this container's antenv has
    no axon_hooks module, but the slim boot package ships the ctypes equivalent.
    """
    try:
        from antenv.axon_hooks import get_axon_ntff_profile_hook  # noqa: F401

        return True
    except ImportError:
        pass
    try:
        import types

        import antenv
        from trn_agent_boot.trn_boot import _ntff_profile_via_ctypes

        hook = _ntff_profile_via_ctypes("/opt/axon/libaxon_pjrt.so")
        if hook is None:
            return False
        mod = types.ModuleType("antenv.axon_hooks")
        state = {"hook": hook}
        mod.set_axon_ntff_profile_hook = lambda h: state.update(hook=h)
        mod.get_axon_ntff_profile_hook = lambda: state["hook"]
        sys.modules["antenv.axon_hooks"] = mod
        antenv.axon_hooks = mod
        return True
    except Exception as e:  # profiling is best-effort; never block execution
        print(f"NTFF hook setup failed ({type(e).__name__}: {e}); no HW timing",
              file=sys.stderr)
        return False


def kernel(x, weight, bias):
    global LAST_EXEC_TIME_NS
    x = np.ascontiguousarray(np.asarray(x, dtype=np.float32))
    weight = np.asarray(weight, dtype=np.float32)
    bias = np.ascontiguousarray(np.asarray(bias, dtype=np.float32))
    wT = np.ascontiguousarray(weight.T)

    in_maps = []
    for c in range(NCORES):
        xT_c = np.ascontiguousarray(x[c * M_LOC : (c + 1) * M_LOC, :].T)
        in_maps.append({"xT": xT_c, "wT": wT, "bias": bias})

    nc = _get_nc()
    trace = os.environ.get("BINLIN_TRACE", "0") == "1"
    if trace:
        trace = _ensure_ntff_hook()
    core_ids = list(range(NCORES))
    if trace:
        try:
            res = run_bass_kernel_spmd(nc, in_maps, core_ids=core_ids, trace=True)
        except Exception as e:
            print(f"traced run failed ({type(e).__name__}: {e}); retry untraced",
                  file=sys.stderr)
            res = run_bass_kernel_spmd(nc, in_maps, core_ids=core_ids, trace=False)
    else:
        res = run_bass_kernel_spmd(nc, in_maps, core_ids=core_ids, trace=False)
    if res.exec_time_ns is not None:
        LAST_EXEC_TIME_NS = res.exec_time_ns

    outT = np.empty((D_OUT, N_TOK), dtype=np.float32)
    for c in range(NCORES):
        outT[:, c * M_LOC : (c + 1) * M_LOC] = res.results[c]["outT"]
    return np.ascontiguousarray(outT.T)
